# revision 1
# baseline (speedup 1.0000x reference)
"""Bass/Trainium2 kernel for nn_CapsuleLayer (dynamic routing capsule layer).

Reference computation:
    inputs: [B=32, J=2048, I=64], W: [K=32, J=2048, D=32, I=64]
    inputs_hat[b,k,j,d] = sum_i inputs[b,j,i] * W[k,j,d,i]
    3 routing iterations (softmax over K), output = squash(s_2)  [B, K, D]

Sharding: J (input capsules) split 8 ways -> J_loc = 256 per core.
Routing softmax (over K) is fully local; only the per-iteration
s[b,k,d] = sum_j c*hat partial sums need a 128KB AllReduce.

Device layouts (per core):
  x stations : [NPAIR=128, 128, 64]  fp16, block-diag pairs (2 j per station)
  W moving   : [NPAIR=128, 128, 1024] fp16 = [pair, (jp,i), (d,k)]
  hat        : SBUF fp16 [128, 64, 1024] = [(jj,b), group, (d,k)]
  s / outputs: [32, 1024] fp32 = [b, (d,k)]
"""

import os
import sys
import numpy as np

import concourse.bass as bass
import concourse.mybir as mybir
import concourse.tile as tile
from concourse import bacc
from concourse import bass_utils

AF = mybir.ActivationFunctionType
ALU = mybir.AluOpType
F16 = mybir.dt.float16
F32 = mybir.dt.float32

EPS = 1e-07
N_CORES = 8
B = 32          # batch
J = 2048        # input capsules (total)
I = 64          # input capsule dim
K = 32          # output capsules
D = 32          # output capsule dim
JL = J // N_CORES          # 256 local input capsules
NPAIR = JL // 2            # 128 station pairs
NGRP = JL // 4             # 64 groups of 4 j's
GPC = 8                    # groups per chunk in routing passes
NCHUNK = NGRP // GPC       # 16 chunks
DK = D * K                 # 1024


def build_program():
    """Build the SPMD bass program (same program on all 8 cores)."""
    nc = bacc.Bacc("TRN2", target_bir_lowering=False, debug=False,
                   enable_asserts=False, num_devices=N_CORES)

    xs = nc.dram_tensor("xs", [NPAIR, 128, I], F16, kind="ExternalInput").ap()
    wt = nc.dram_tensor("wt", [NPAIR, 128, DK], F16, kind="ExternalInput").ap()
    diag = nc.dram_tensor("diag", [128, B], F16, kind="ExternalInput").ap()
    out_d = nc.dram_tensor("out", [B, DK], F32, kind="ExternalOutput").ap()

    with tile.TileContext(nc) as tc:
        _emit(tc, xs, wt, diag, out_d)
    nc.compile()
    return nc


def _emit(tc, xs, wt, diag, out_d):
    nc = tc.nc
    with (
        tc.tile_pool(name="hat", bufs=1) as hat_pool,
        tc.tile_pool(name="wld", bufs=3) as w_pool,
        tc.tile_pool(name="xld", bufs=2) as x_pool,
        tc.tile_pool(name="big", bufs=2) as big_pool,       # prod/ch chunk tiles
        tc.tile_pool(name="tree", bufs=1) as tree_pool,
        tc.tile_pool(name="smx", bufs=1) as smx_pool,
        tc.tile_pool(name="small", bufs=1) as small_pool,
        tc.tile_pool(name="obc", bufs=1) as obc_pool,
        tc.tile_pool(name="const", bufs=1) as const_pool,
        tc.tile_pool(name="accps", bufs=1, space="PSUM") as acc_psum,
        tc.tile_pool(name="hatps", bufs=3, space="PSUM") as hat_psum,
        tc.tile_pool(name="dram", bufs=6, space="DRAM") as dram_pool,
    ):
        # ---- constants ----
        diag_sb = const_pool.tile([128, B], F16, tag="diag")
        nc.sync.dma_start(diag_sb[:], diag)

        # persistent hat storage: [(jj,b), group, (d,k)] fp16
        hat_sb = hat_pool.tile([128, NGRP, DK], F16, tag="hat")

        # O accumulator (sum of squash outputs over past iterations)
        o_acc = const_pool.tile([B, DK], F32, tag="oacc")

        # ---- Pass A: hat = x @ W; s0 accumulated on DVE from PSUM tiles ----
        s0_acc = const_pool.tile([128, DK], F32, tag="s0acc")
        diag32 = const_pool.tile([128, B], F32, tag="diag32")
        nc.vector.tensor_copy(diag32[:], diag_sb[:])
        for g in range(NGRP):
            xg = x_pool.tile([128, 2, I], F16, tag="x")
            nc.sync.dma_start(xg[:], xs[2 * g: 2 * g + 2].rearrange("q p f -> p q f"))

            ps = hat_psum.tile([128, DK], F32, tag="hatps", name=f"hat_ps{g}")
            for q in (0, 1):            # station pair within group
                wq = w_pool.tile([128, DK], F16, tag="w")
                nc.sync.dma_start(wq[:], wt[2 * g + q])
                for jp in (0, 1):       # j within pair: row-half jp*64
                    jj = q * 2 + jp
                    for h in (0, 1):    # free-dim half
                        nc.tensor.matmul(
                            ps[jj * 32:(jj + 1) * 32, h * 512:(h + 1) * 512],
                            lhsT=xg[jp * 64:(jp + 1) * 64, q, jp * 32:(jp + 1) * 32],
                            rhs=wq[jp * 64:(jp + 1) * 64, h * 512:(h + 1) * 512],
                            start=True, stop=True,
                            tile_position=(jp * 64, jj * 32),
                        )
            # PSUM -> SBUF fp16 (split across ScalarE / VectorE)
            nc.scalar.copy(hat_sb[:, g, 0:512], ps[:, 0:512])
            nc.vector.tensor_copy(hat_sb[:, g, 512:DK], ps[:, 512:DK])
            # s0 accumulation on DVE (PE stays free)
            if g == 0:
                nc.vector.tensor_copy(s0_acc[:], ps[:])
            else:
                nc.vector.tensor_add(s0_acc[:], s0_acc[:], ps[:])
        # jj-sum of s0_acc via 2 diagonal matmuls
        s_ps = acc_psum.tile([128, DK], F32, tag="sacc", name="s0_ps")
        for h in (0, 1):
            nc.tensor.matmul(
                s_ps[0:B, h * 512:(h + 1) * 512],
                lhsT=diag32[:],
                rhs=s0_acc[:, h * 512:(h + 1) * 512],
                start=True, stop=True,
                skip_group_check=True,
            )

        # ---- routing iterations ----
        for r in range(3):
            # s partial -> AllReduce -> s_full
            s_loc = small_pool.tile([B, DK], F32, tag="sloc", name=f"s_loc{r}")
            nc.vector.tensor_copy(s_loc[:], s_ps[0:B, :])
            if r > 0:
                nc.vector.tensor_add(s_loc[:], s_loc[:], s_ps[B:2 * B, :])
                nc.vector.tensor_add(s_loc[:], s_loc[:], s_ps[2 * B:3 * B, :])
                nc.vector.tensor_add(s_loc[:], s_loc[:], s_ps[3 * B:4 * B, :])
            s16 = small_pool.tile([B, DK], F16, tag="s16", name=f"s16_{r}")
            nc.vector.tensor_copy(s16[:], s_loc[:])
            ar_in = dram_pool.tile([B, DK], F16, name=f"ar_in{r}")
            ar_out = dram_pool.tile([B, DK], F16, name=f"ar_out{r}")
            nc.sync.dma_start(ar_in[:], s16[:])
            nc.gpsimd.collective_compute(
                "AllReduce", ALU.add,
                replica_groups=[list(range(N_CORES))],
                ins=[ar_in.opt()],
                outs=[ar_out.opt()],
            )
            s_full = small_pool.tile([B, DK], F32, tag="sfull", name=f"s_full{r}")
            nc.sync.dma_start(s16[:], ar_out[:])
            nc.vector.tensor_copy(s_full[:], s16[:])
            if r == 0:
                nc.vector.tensor_scalar_mul(s_full[:], s_full[:], 1.0 / K)

            # squash: scale = s2/(1+s2)/sqrt(s2+eps), per (b,k); s2 = sum_d s^2
            sq = s_loc
            nc.scalar.square(sq[:], s_full[:])
            s2 = small_pool.tile([B, K], F32, tag="s2")
            nc.vector.reduce_sum(s2[:], sq.rearrange("p (d k) -> p k d", d=D),
                                 axis=mybir.AxisListType.X)
            t2 = small_pool.tile([B, K], F32, tag="t2")
            nc.vector.tensor_scalar_add(t2[:], s2[:], EPS)
            nc.scalar.sqrt(t2[:], t2[:])
            t1 = small_pool.tile([B, K], F32, tag="t1")
            nc.vector.scalar_tensor_tensor(t1[:], s2[:], 1.0, t2[:],
                                           ALU.add, ALU.mult)
            nc.vector.reciprocal(t1[:], t1[:])
            nc.vector.tensor_mul(s2[:], s2[:], t1[:])         # scale [B, K]
            o_r = s_full
            nc.vector.tensor_tensor(
                o_r.rearrange("p (d k) -> p d k", d=D),
                s_full.rearrange("p (d k) -> p d k", d=D),
                s2[:, None, :].to_broadcast([B, D, K]),
                ALU.mult,
            )

            if r == 2:
                nc.sync.dma_start(out_d, o_r[:])
                break

            # O_acc += o_r ; build O_bcast fp16 [128, (d,k)]
            if r == 0:
                nc.vector.tensor_copy(o_acc[:], o_r[:])
            else:
                nc.vector.tensor_add(o_acc[:], o_acc[:], o_r[:])
            o16 = small_pool.tile([B, DK], F16, tag="o16", name=f"o16_{r}")
            nc.vector.tensor_copy(o16[:], o_acc[:])
            o_bc = obc_pool.tile([128, DK], F16, tag="obc", name=f"obc_{r}")
            for jj in range(4):
                nc.sync.dma_start(o_bc[jj * 32:(jj + 1) * 32, :], o16[:])

            # next-iteration s accumulator
            s_ps = acc_psum.tile([128, DK], F32, tag="sacc", name=f"s{r + 1}_ps")

            # routing pass over hat chunks
            for ci in range(NCHUNK):
                gsl = slice(ci * GPC, (ci + 1) * GPC)
                hat_c = hat_sb[:, gsl, :]
                # u = sum_d hat * O_acc   (fp16 mul + pairwise tree over d)
                prod = big_pool.tile([128, GPC, DK], F16, tag="big",
                                     name=f"prod_{r}_{ci}")
                nc.vector.tensor_tensor(
                    prod[:], hat_c,
                    o_bc[:, None, :].to_broadcast([128, GPC, DK]),
                    ALU.mult,
                )
                p4 = prod.rearrange("p g (d k) -> p g d k", d=D)
                nc.vector.tensor_add(p4[:, :, 0:16, :], p4[:, :, 0:16, :],
                                     p4[:, :, 16:32, :])
                nc.vector.tensor_add(p4[:, :, 0:8, :], p4[:, :, 0:8, :],
                                     p4[:, :, 8:16, :])
                nc.vector.tensor_add(p4[:, :, 0:4, :], p4[:, :, 0:4, :],
                                     p4[:, :, 4:8, :])
                t2t = tree_pool.tile([128, GPC, 2, K], F32, tag="t2")
                nc.vector.tensor_add(t2t[:], p4[:, :, 0:2, :], p4[:, :, 2:4, :])
                u = smx_pool.tile([128, GPC, K], F32, tag="u")
                nc.vector.tensor_add(u[:], t2t[:, :, 0, :], t2t[:, :, 1, :])

                # softmax over k (free dim); u is bounded (|O|<=2), skip max-sub
                nc.scalar.activation(u[:], u[:], AF.Exp)
                z = smx_pool.tile([128, GPC], F32, tag="z")
                nc.vector.reduce_sum(z[:], u[:], axis=mybir.AxisListType.X)
                nc.vector.reciprocal(z[:], z[:])
                c16 = smx_pool.tile([128, GPC, K], F16, tag="c16")
                nc.vector.tensor_tensor(
                    c16[:], u[:], z[:, :, None].to_broadcast([128, GPC, K]),
                    ALU.mult,
                )

                # ch = c * hat ; PE partition-sum into s_ps
                ch = big_pool.tile([128, GPC, DK], F16, tag="big",
                                   name=f"ch_{r}_{ci}")
                nc.vector.tensor_tensor(
                    ch.rearrange("p g (d k) -> p g d k", d=D),
                    hat_c.rearrange("p g (d k) -> p g d k", d=D),
                    c16[:, :, None, :].to_broadcast([128, GPC, D, K]),
                    ALU.mult,
                )
                for gg in range(GPC):
                    gglob = ci * GPC + gg
                    c = gglob % 4
                    for h in (0, 1):
                        nc.tensor.matmul(
                            s_ps[32 * c:32 * (c + 1), h * 512:(h + 1) * 512],
                            lhsT=diag_sb[:],
                            rhs=ch[:, gg, h * 512:(h + 1) * 512],
                            start=(gglob < 4),
                            stop=(gglob >= NGRP - 4),
                            tile_position=(0, 32 * c),
                            skip_group_check=True,
                        )


def pack_inputs(inputs, W):
    """Host-side shard + layout pack. Returns in_maps (one dict per core)."""
    diag = np.zeros((128, B), np.float16)
    for p in range(128):
        diag[p, p % B] = 1.0

    # W: [K, J, D, I] -> per core [JL, I, D, K] fp16 -> [NPAIR, 128, DK]
    in_maps = []
    for c in range(N_CORES):
        jsl = slice(c * JL, (c + 1) * JL)
        wc = np.ascontiguousarray(
            W[:, jsl].transpose(1, 3, 2, 0), dtype=np.float16
        )  # [JL, I, D, K]
        wt = wc.reshape(NPAIR, 2 * I, DK)

        xc = inputs[:, jsl, :]  # [B, JL, I]
        xs = np.zeros((NPAIR, 128, I), np.float16)
        xt = np.ascontiguousarray(xc.transpose(1, 2, 0))  # [JL, I, B]
        xs[:, 0:I, 0:B] = xt[0::2]
        xs[:, I:128, B:2 * B] = xt[1::2]
        in_maps.append({"xs": xs, "wt": wt, "diag": diag})
    return in_maps


_CACHED_NC = None


def _install_ntff_hook():
    """Provide antenv.axon_hooks.get_axon_ntff_profile_hook when the agent
    image lacks it, by driving the injected libaxon_pjrt.so directly
    (mirrors trn_agent_boot._ntff_profile_via_ctypes)."""
    import types
    import ctypes
    import contextlib
    try:
        from antenv.axon_hooks import get_axon_ntff_profile_hook  # noqa: F401
        return True
    except ImportError:
        pass
    so_path = "/opt/axon/libaxon_pjrt.so"
    if not os.path.exists(so_path):
        return False
    lib = ctypes.CDLL(so_path)
    if not hasattr(lib, "axon_start_nrt_profile"):
        return False
    lib.axon_start_nrt_profile.argtypes = [
        ctypes.POINTER(ctypes.c_int64), ctypes.c_size_t]
    lib.axon_start_nrt_profile.restype = ctypes.c_int64
    lib.axon_stop_nrt_profile.argtypes = [ctypes.c_char_p]
    lib.axon_stop_nrt_profile.restype = ctypes.c_int64

    @contextlib.contextmanager
    def _hook(output_dir, device_ids):
        import jax
        jax.devices()
        if device_ids:
            ids = (ctypes.c_int64 * len(device_ids))(*device_ids)
            rc = lib.axon_start_nrt_profile(ids, len(device_ids))
        else:
            rc = lib.axon_start_nrt_profile(None, 0)
        if rc != 0:
            raise RuntimeError(f"axon_start_nrt_profile rc={rc}")
        try:
            yield
        finally:
            n = lib.axon_stop_nrt_profile(str(output_dir).encode())
            if n < 0:
                raise RuntimeError(f"axon_stop_nrt_profile rc={n}")

    import antenv
    mod = types.ModuleType("antenv.axon_hooks")
    mod.get_axon_ntff_profile_hook = lambda: _hook
    mod.set_axon_ntff_profile_hook = lambda h: None
    sys.modules["antenv.axon_hooks"] = mod
    antenv.axon_hooks = mod
    return True


def kernel(inputs, W):
    global _CACHED_NC
    inputs = np.asarray(inputs)
    W = np.asarray(W)
    if _CACHED_NC is None:
        _CACHED_NC = build_program()
    nc = _CACHED_NC
    in_maps = pack_inputs(inputs, W)
    trace = bool(int(os.environ.get("CAPS_TRACE", "0")))
    if trace:
        trace = _install_ntff_hook()
    res = bass_utils.run_bass_kernel_spmd(
        nc, in_maps, core_ids=list(range(N_CORES)), trace=trace,
    )
    kernel.last_results = res
    if trace and res.exec_time_ns is not None:
        print(f"HW exec time: {res.exec_time_ns} ns", file=sys.stderr)
        kernel.last_exec_time_ns = res.exec_time_ns
    out = res.results[0]["out"]  # [B, DK] fp32, identical on all cores
    return np.ascontiguousarray(
        out.reshape(B, D, K).transpose(0, 2, 1)
    ).astype(np.float32)


kernel.last_exec_time_ns = None
kernel.last_results = None



# revision 15
# speedup vs baseline: 1.0121x; 1.0121x over previous
"""Bass/Trainium2 kernel for nn_CapsuleLayer (dynamic routing capsule layer).

Reference computation:
    inputs: [B=32, J=2048, I=64], W: [K=32, J=2048, D=32, I=64]
    hat[b,j,d,k] = sum_i inputs[b,j,i] * W[k,j,d,i]
    3 routing iterations (softmax over K), output = squash(s_2)  [B, K, D]

Sharding: J (input capsules) split 8 ways -> JL = 256 per core.
Per-iteration s[b,k,d] partial sums need a 64KB fp16 AllReduce.

v2 design (vs the DVE-bound v1):
  - Pass A runs W-STATIONARY: lhsT = W chunk [(jp,i)=128, dk=128] (gets FWL),
    rhs = block-diag x station [128, 64] -> hat lands in "hat3" layout
    [p=(dm4,k), chunk c, b, j] with d = 4c + dm4.  PE cost halves and the
    (d,k) axis lands in PARTITIONS.
  - u[b,k,j] = sum_d O[b,k,d] hat: done ON PE with delta-masked stations
    osta[p,k'] = (p%32==k') * O[p-layout], accumulated over the 8 dk-chunks.
    4 batches b share one PSUM tile via tile_position col groups.
  - softmax over k stays in k-in-partition form: exp on ScalarE (shifted to
    keep fp16 range), Z = sum_k via a block-ones PE matmul, 1/Z on DVE.
  - Only c (x) hat multiply + the j-reduction tree remain on DVE/Pool.
  - s0 = sum_j hat computed with DVE reduce_sum under the pass-A DMA shadow.
  - A tiny warm-up AllReduce at t=0 absorbs the collective rendezvous cost.
"""

import os
import sys
import numpy as np

import concourse.bass as bass
import concourse.mybir as mybir
import concourse.tile as tile
from concourse import bacc
from concourse import bass_utils

AF = mybir.ActivationFunctionType
ALU = mybir.AluOpType
F16 = mybir.dt.float16
F32 = mybir.dt.float32

EPS = 1e-07
N_CORES = 8
B = 32          # batch
J = 2048        # input capsules (total)
I = 64          # input capsule dim
K = 32          # output capsules
D = 32          # output capsule dim
JL = J // N_CORES          # 256 local input capsules
NPAIR = JL // 2            # 128 station pairs
DK = D * K                 # 1024
NC = DK // 128             # 8 dk-chunks of 128
EXP_SHIFT = -5.0           # softmax logit shift so exp() fits fp16 range


def build_program():
    """Build the SPMD bass program (same program on all 8 cores)."""
    nc = bacc.Bacc("TRN2", target_bir_lowering=False, debug=False,
                   enable_asserts=False, num_devices=N_CORES)

    xs = nc.dram_tensor("xs", [128, NPAIR, I], F16, kind="ExternalInput").ap()
    wt = nc.dram_tensor("wt", [NPAIR, 128, DK], F16, kind="ExternalInput").ap()
    mask = nc.dram_tensor("mask", [128, K], F16, kind="ExternalInput").ap()
    zsta = nc.dram_tensor("zsta", [128, 128], F16, kind="ExternalInput").ap()
    rep32 = nc.dram_tensor("rep32", [K, 128], F32, kind="ExternalInput").ap()
    crep = nc.dram_tensor("crep", [128, 4, 128], F16,
                          kind="ExternalInput").ap()
    out_d = nc.dram_tensor("out", [128, NC * B], F32,
                           kind="ExternalOutput").ap()

    with tile.TileContext(nc) as tc:
        _emit(tc, xs, wt, mask, zsta, rep32, crep, out_d)
    nc.compile()
    return nc


def _emit(tc, xs_d, wt_d, mask_d, zsta_d, rep32_d, crep_d, out_d):
    nc = tc.nc
    with (
        tc.tile_pool(name="hat", bufs=1) as hat_pool,
        tc.tile_pool(name="cst", bufs=1) as const_pool,
        tc.tile_pool(name="wld", bufs=4) as w_pool,
        tc.tile_pool(name="xld", bufs=2) as x_pool,
        tc.tile_pool(name="chp", bufs=2) as ch_pool,
        tc.tile_pool(name="ep", bufs=3) as e_pool,
        tc.tile_pool(name="rzp", bufs=2) as rz_pool,
        tc.tile_pool(name="ckp", bufs=2) as ck_pool,
        tc.tile_pool(name="sm", bufs=1) as small_pool,
        tc.tile_pool(name="s16p", bufs=1) as s16_pool,
        tc.tile_pool(name="psA", bufs=2, space="PSUM") as psA_pool,
        tc.tile_pool(name="psR", bufs=6, space="PSUM") as psR_pool,
        tc.tile_pool(name="dram", bufs=8, space="DRAM") as dram_pool,
    ):
        # ---- constants ----
        mask_sb = const_pool.tile([128, K], F16, tag="mask")
        nc.sync.dma_start(mask_sb[:], mask_d)
        zsta_sb = const_pool.tile([128, 128], F16, tag="zsta")
        nc.sync.dma_start(zsta_sb[:], zsta_d)
        rep32_sb = const_pool.tile([K, 128], F32, tag="rep32")
        nc.sync.dma_start(rep32_sb[:], rep32_d)
        crep_sb = const_pool.tile([128, 4, 128], F16, tag="crep")
        nc.sync.dma_start(crep_sb[:], crep_d)
        ebias = const_pool.tile([128, 1], F32, tag="ebias")
        nc.vector.memset(ebias[:], EXP_SHIFT)
        mask32 = const_pool.tile([128, K], F32, tag="mask32")
        nc.vector.tensor_copy(mask32[:], mask_sb[:])

        # warm up the collective path early (absorbs CC rendezvous latency)
        warm_in = dram_pool.tile([128, K], F16, name="warm_in")
        warm_out = dram_pool.tile([128, K], F16, name="warm_out")
        nc.sync.dma_start(warm_in[:], mask_d)
        nc.gpsimd.collective_compute(
            "AllReduce", ALU.add,
            replica_groups=[list(range(N_CORES))],
            ins=[warm_in.opt()],
            outs=[warm_out.opt()],
        )

        # persistent tensors
        # hat3[p=(dm4,k), c, b, j] = hat[b, j, dk=c*128+p]  (d = 4c + dm4)
        hat3 = hat_pool.tile([128, NC, B, JL], F16, tag="hat")
        c_exp = const_pool.tile([128, B, JL], F16, tag="cexp")
        osta = const_pool.tile([128, NC, B, K], F16, tag="osta")
        o_acc = const_pool.tile([128, NC, B], F32, tag="oacc")
        o_acc16 = const_pool.tile([128, NC, B], F16, tag="oacc16")
        s0h = const_pool.tile([128, NC, B, 2], F32, tag="s0h")

        # ---- Pass A: hat = x @ W (W stationary), s0 = sum_j hat ----
        # copies psum->sbuf on ScalarE(5/8) + Pool(3/8); DVE does s0 reduces
        for pr in range(NPAIR):
            if pr % 16 == 0:
                xs_t = x_pool.tile([128, 16, I], F16, tag="xs",
                                   name=f"xs_{pr}")
                nc.sync.dma_start(xs_t[:], xs_d[:, pr:pr + 16, :])
            wq = w_pool.tile([128, DK], F16, tag="w", name=f"w_{pr}")
            nc.sync.dma_start(wq[:], wt_d[pr])
            ps = psA_pool.tile([128, NC, 2 * B], F32, tag="psA",
                               name=f"psA_{pr}")
            for c in range(NC):
                nc.tensor.matmul(
                    ps[:, c, :],
                    lhsT=wq[:, c * 128:(c + 1) * 128],
                    rhs=xs_t[:, pr % 16, :],
                    start=True, stop=True,
                )
            src = ps.rearrange("p c (b q) -> p c b q", q=2)
            dst = hat3[:, :, :, 2 * pr:2 * pr + 2]
            nc.scalar.copy(dst, src)  # GPSIMD can't read PSUM; Act does all
            # s0 partial reduces once each j-half is complete
            if pr == NPAIR // 2 - 1 or pr == NPAIR - 1:
                h = 0 if pr < NPAIR // 2 else 1
                for bb in range(16):
                    bs = 2 * bb
                    nc.vector.reduce_sum(
                        s0h[:, :, bs:bs + 2, h:h + 1],
                        hat3[:, :, bs:bs + 2, 128 * h:128 * (h + 1)],
                        axis=mybir.AxisListType.X,
                    )

        s16 = s16_pool.tile([128, NC * B], F16, tag="s16", name="s16_0")
        nc.vector.tensor_tensor(
            s16.rearrange("p (c b) -> p c b", c=NC),
            s0h[:, :, :, 0], s0h[:, :, :, 1], ALU.add,
        )

        # ---- routing iterations ----
        for r in range(3):
            # AllReduce the s partial sums (fp16, 64KB)
            ar_in = dram_pool.tile([128, NC * B], F16, name=f"ar_in{r}")
            ar_out = dram_pool.tile([128, NC * B], F16, name=f"ar_out{r}")
            nc.sync.dma_start(ar_in[:], s16[:])
            nc.gpsimd.collective_compute(
                "AllReduce", ALU.add,
                replica_groups=[list(range(N_CORES))],
                ins=[ar_in.opt()],
                outs=[ar_out.opt()],
            )
            s16b = s16_pool.tile([128, NC * B], F16, tag="s16b",
                                 name=f"s16b{r}")
            nc.sync.dma_start(s16b[:], ar_out[:])
            s_full = small_pool.tile([128, NC, B], F32, tag="sfull",
                                     name=f"sfull{r}")
            sfv = s_full.rearrange("p c b -> p (c b)")
            if r == 0:
                nc.vector.tensor_scalar_mul(sfv, s16b[:], 1.0 / K)
            else:
                nc.vector.tensor_copy(sfv, s16b[:])

            # squash scale: s2/(1+s2)/sqrt(s2+eps) per (k, b)
            sq = small_pool.tile([128, NC, B], F32, tag="sq", name=f"sq{r}")
            nc.scalar.square(sq[:], s_full[:])
            t4 = small_pool.tile([128, 4, B], F32, tag="t4", name=f"t4_{r}")
            nc.vector.tensor_tensor(t4[:], sq[:, 0:4, :], sq[:, 4:8, :],
                                    ALU.add)
            t2 = small_pool.tile([128, 2, B], F32, tag="t2", name=f"t2_{r}")
            nc.vector.tensor_tensor(t2[:], t4[:, 0:2, :], t4[:, 2:4, :],
                                    ALU.add)
            t1 = small_pool.tile([128, B], F32, tag="t1", name=f"t1_{r}")
            nc.vector.tensor_tensor(t1[:], t2[:, 0, :], t2[:, 1, :], ALU.add)
            # fold over the 4 dm4 partition groups via a delta-station matmul
            sqz = psR_pool.tile([128, JL], F32, tag="psr", name=f"sqz{r}")
            nc.tensor.matmul(sqz[0:K, 0:B], lhsT=mask32[:], rhs=t1[:],
                             start=True, stop=True)
            s2e = small_pool.tile([K, B], F32, tag="s2e", name=f"s2e{r}")
            nc.vector.tensor_scalar_add(s2e[:], sqz[0:K, 0:B], EPS)
            rt = small_pool.tile([K, B], F32, tag="rt", name=f"rt{r}")
            nc.scalar.sqrt(rt[:], s2e[:])
            den = small_pool.tile([K, B], F32, tag="den", name=f"den{r}")
            nc.vector.scalar_tensor_tensor(den[:], sqz[0:K, 0:B], 1.0, rt[:],
                                           ALU.add, ALU.mult)
            rden = small_pool.tile([K, B], F32, tag="rden", name=f"rden{r}")
            nc.vector.reciprocal_approx_fast(rden[:], den[:])
            scl = small_pool.tile([K, B], F32, tag="scl", name=f"scl{r}")
            nc.vector.tensor_tensor(scl[:], sqz[0:K, 0:B], rden[:], ALU.mult)
            # replicate scl to all 128 partitions via delta station
            sclp = psR_pool.tile([128, JL], F32, tag="psr", name=f"sclp{r}")
            nc.tensor.matmul(sclp[:, 0:B], lhsT=rep32_sb[:], rhs=scl[:],
                             start=True, stop=True)
            scl128 = small_pool.tile([128, B], F32, tag="sc128",
                                     name=f"sc128_{r}")
            nc.scalar.copy(scl128[:], sclp[:, 0:B])

            o_r = small_pool.tile([128, NC, B], F32, tag="or", name=f"or{r}")
            nc.vector.tensor_tensor(
                o_r[:],
                s_full[:],
                scl128[:, None, :].to_broadcast([128, NC, B]),
                ALU.mult,
            )

            if r == 2:
                nc.sync.dma_start(out_d, o_r.rearrange("p c b -> p (c b)"))
                break

            if r == 0:
                nc.vector.tensor_copy(o_acc[:], o_r[:])
            else:
                nc.vector.tensor_add(o_acc[:], o_acc[:], o_r[:])
            nc.scalar.copy(o_acc16[:], o_acc[:])
            # delta-masked stations: osta[p,c,b,k'] = (p%32==k') * O_acc[p,c,b]
            nc.gpsimd.tensor_tensor(
                osta[:],
                mask_sb[:, None, None, :].to_broadcast([128, NC, B, K]),
                o_acc16[:, :, :, None].to_broadcast([128, NC, B, K]),
                ALU.mult,
            )

            s16 = s16_pool.tile([128, NC * B], F16, tag="s16",
                                name=f"s16_{r + 1}")
            s16v = s16.rearrange("p (c b) -> p c b", c=NC)

            # routing pass over 4-b blocks
            chi = 0
            for bb4 in range(8):
                b0 = 4 * bb4
                u_ps = psR_pool.tile([128, JL], F32, tag="psr",
                                     name=f"ups{r}_{bb4}")
                for c in range(NC):
                    for g in range(4):
                        nc.tensor.matmul(
                            u_ps[32 * g:32 * (g + 1), :],
                            lhsT=osta[:, c, b0 + g, :],
                            rhs=hat3[:, c, b0 + g, :],
                            start=(c == 0), stop=(c == NC - 1),
                            tile_position=(0, 32 * g),
                            skip_group_check=True,
                        )
                e16 = e_pool.tile([128, JL], F16, tag="e16",
                                  name=f"e{r}_{bb4}")
                nc.scalar.activation(e16[:], u_ps[:], AF.Exp, bias=ebias[:])
                z_ps = psR_pool.tile([128, JL], F32, tag="psr",
                                     name=f"z{r}_{bb4}")
                nc.tensor.matmul(z_ps[:], lhsT=zsta_sb[:], rhs=e16[:],
                                 start=True, stop=True)
                rz32 = rz_pool.tile([128, JL], F32, tag="rz32",
                                    name=f"rz32_{r}_{bb4}")
                nc.vector.reciprocal_approx_fast(rz32[:], z_ps[:])
                rz16 = rz_pool.tile([128, JL], F16, tag="rz16",
                                    name=f"rz16_{r}_{bb4}")
                nc.scalar.copy(rz16[:], rz32[:])
                # softmax weights, k-in-partition, aligned per b-group
                c_k4 = ck_pool.tile([128, JL], F16, tag="ck",
                                    name=f"ck{r}_{bb4}")
                for g in range(4):
                    nc.vector.tensor_tensor(
                        c_k4[32 * g:32 * (g + 1), :],
                        e16[32 * g:32 * (g + 1), :],
                        rz16[32 * g:32 * (g + 1), :],
                        ALU.mult,
                    )
                # replicate each b's [k, j] block to all 128 partitions (PE)
                for g in range(4):
                    ce = psR_pool.tile([128, JL], F32, tag="psr",
                                       name=f"ce{r}_{bb4}_{g}")
                    nc.tensor.matmul(ce[:], lhsT=crep_sb[:, g, :],
                                     rhs=c_k4[:], start=True, stop=True)
                    nc.scalar.copy(c_exp[:, b0 + g, :], ce[:])
                # ch = c*hat, then j-reduction tree (in place), 2-b blocks
                for sb in range(2):
                    bs = b0 + 2 * sb
                    eng = nc.gpsimd if chi % 4 == 3 else nc.vector
                    chi += 1
                    ch = ch_pool.tile([128, NC, 2, JL], F16, tag="ch",
                                      name=f"ch{r}_{bs}")
                    eng.tensor_tensor(
                        ch[:],
                        hat3[:, :, bs:bs + 2, :],
                        c_exp[:, None, bs:bs + 2, :].to_broadcast(
                            [128, NC, 2, JL]),
                        ALU.mult,
                    )
                    w = 128
                    while w >= 2:
                        eng.tensor_tensor(ch[:, :, :, 0:w], ch[:, :, :, 0:w],
                                          ch[:, :, :, w:2 * w], ALU.add)
                        w //= 2
                    eng.tensor_tensor(s16v[:, :, bs:bs + 2],
                                      ch[:, :, :, 0], ch[:, :, :, 1],
                                      ALU.add)


def pack_inputs(inputs, W):
    """Host-side shard + layout pack. Returns in_maps (one dict per core)."""
    mask = np.zeros((128, K), np.float16)
    mask[np.arange(128), np.arange(128) % K] = 1.0
    zsta = np.kron(np.eye(4, dtype=np.float16),
                   np.ones((32, 32), np.float16))
    # rep32[k, m] = (m%32 == k): replicates a [32, .] tile to 128 partitions
    rep32 = np.zeros((K, 128), np.float32)
    rep32[np.arange(128) % K, np.arange(128)] = 1.0
    # crep[p', g, m] = (p'//32 == g) & (p'%32 == m%32): selects b-group g's
    # [k, j] block and replicates it across the 4 dm4 partition groups
    crep = np.zeros((128, 4, 128), np.float16)
    pp = np.arange(128)
    for g in range(4):
        sel = (pp // 32 == g)
        for m in range(128):
            crep[sel & (pp % 32 == m % 32), g, m] = 1.0

    in_maps = []
    for c in range(N_CORES):
        jsl = slice(c * JL, (c + 1) * JL)
        # W: [K, J, D, I] -> [JL, I, D, K] -> [pair, (jp,i), (d,k)] fp16
        wc = np.ascontiguousarray(
            W[:, jsl].transpose(1, 3, 2, 0), dtype=np.float16
        )  # [JL, I, D, K]
        wt = wc.reshape(NPAIR, 2 * I, DK)

        # x stations: xs[p=(jp,i), pair, col=2b+jp] block-diag, partition-major
        xc = inputs[:, jsl, :]  # [B, JL, I]
        xt = np.ascontiguousarray(xc.transpose(1, 2, 0)).astype(np.float16)
        xs = np.zeros((NPAIR, 128, I), np.float16)
        xs[:, 0:I, 0::2] = xt[0::2]      # jp=0 rows, even cols
        xs[:, I:128, 1::2] = xt[1::2]    # jp=1 rows, odd cols
        xs2 = np.ascontiguousarray(xs.transpose(1, 0, 2))  # [128, NPAIR, I]
        in_maps.append({"xs": xs2, "wt": wt, "mask": mask, "zsta": zsta,
                        "rep32": rep32, "crep": crep})
    return in_maps


_CACHED_NC = None


def _install_ntff_hook():
    """Provide antenv.axon_hooks.get_axon_ntff_profile_hook when the agent
    image lacks it, by driving the injected libaxon_pjrt.so directly
    (mirrors trn_agent_boot._ntff_profile_via_ctypes)."""
    import types
    import ctypes
    import contextlib
    try:
        from antenv.axon_hooks import get_axon_ntff_profile_hook  # noqa: F401
        return True
    except ImportError:
        pass
    so_path = "/opt/axon/libaxon_pjrt.so"
    if not os.path.exists(so_path):
        return False
    lib = ctypes.CDLL(so_path)
    if not hasattr(lib, "axon_start_nrt_profile"):
        return False
    lib.axon_start_nrt_profile.argtypes = [
        ctypes.POINTER(ctypes.c_int64), ctypes.c_size_t]
    lib.axon_start_nrt_profile.restype = ctypes.c_int64
    lib.axon_stop_nrt_profile.argtypes = [ctypes.c_char_p]
    lib.axon_stop_nrt_profile.restype = ctypes.c_int64

    @contextlib.contextmanager
    def _hook(output_dir, device_ids):
        import jax
        jax.devices()
        if device_ids:
            ids = (ctypes.c_int64 * len(device_ids))(*device_ids)
            rc = lib.axon_start_nrt_profile(ids, len(device_ids))
        else:
            rc = lib.axon_start_nrt_profile(None, 0)
        if rc != 0:
            raise RuntimeError(f"axon_start_nrt_profile rc={rc}")
        try:
            yield
        finally:
            n = lib.axon_stop_nrt_profile(str(output_dir).encode())
            if n < 0:
                raise RuntimeError(f"axon_stop_nrt_profile rc={n}")

    import antenv
    mod = types.ModuleType("antenv.axon_hooks")
    mod.get_axon_ntff_profile_hook = lambda: _hook
    mod.set_axon_ntff_profile_hook = lambda h: None
    sys.modules["antenv.axon_hooks"] = mod
    antenv.axon_hooks = mod
    return True


def kernel(inputs, W):
    global _CACHED_NC
    inputs = np.asarray(inputs)
    W = np.asarray(W)
    if _CACHED_NC is None:
        _CACHED_NC = build_program()
    nc = _CACHED_NC
    in_maps = pack_inputs(inputs, W)
    trace = bool(int(os.environ.get("CAPS_TRACE", "0")))
    if trace:
        trace = _install_ntff_hook()
    res = bass_utils.run_bass_kernel_spmd(
        nc, in_maps, core_ids=list(range(N_CORES)), trace=trace,
    )
    kernel.last_results = res
    if trace and res.exec_time_ns is not None:
        print(f"HW exec time: {res.exec_time_ns} ns", file=sys.stderr)
        kernel.last_exec_time_ns = res.exec_time_ns
    out = res.results[0]["out"]  # [128 p=(dm4,k), NC*B] fp32 device layout
    a = out.reshape(4, K, NC, B)         # [dm4, k, c, b]; d = 4c + dm4
    return np.ascontiguousarray(
        a.transpose(3, 1, 2, 0).reshape(B, K, D)
    ).astype(np.float32)


kernel.last_exec_time_ns = None
kernel.last_results = None


# revision 27
# speedup vs baseline: 1.0905x; 1.0775x over previous
"""Bass/Trainium2 kernel for nn_CapsuleLayer (dynamic routing capsule layer).

Reference computation:
    inputs: [B=32, J=2048, I=64], W: [K=32, J=2048, D=32, I=64]
    hat[b,j,d,k] = sum_i inputs[b,j,i] * W[k,j,d,i]
    3 routing iterations (softmax over K), output = squash(s_2)  [B, K, D]

Sharding: J (input capsules) split 8 ways -> JL = 256 per core.
Per-iteration s[b,k,d] partial sums need a 64KB fp16 AllReduce.

v2 design (vs the DVE-bound v1):
  - Pass A runs W-STATIONARY: lhsT = W chunk [(jp,i)=128, dk=128] (gets FWL),
    rhs = block-diag x station [128, 64] -> hat lands in "hat3" layout
    [p=(dm4,k), chunk c, b, j] with d = 4c + dm4.  PE cost halves and the
    (d,k) axis lands in PARTITIONS.
  - u[b,k,j] = sum_d O[b,k,d] hat: done ON PE with delta-masked stations
    osta[p,k'] = (p%32==k') * O[p-layout], accumulated over the 8 dk-chunks.
    4 batches b share one PSUM tile via tile_position col groups.
  - softmax over k stays in k-in-partition form: exp on ScalarE (shifted to
    keep fp16 range), Z = sum_k via a block-ones PE matmul, 1/Z on DVE.
  - Only c (x) hat multiply + the j-reduction tree remain on DVE/Pool.
  - s0 = sum_j hat computed with DVE reduce_sum under the pass-A DMA shadow.
  - A tiny warm-up AllReduce at t=0 absorbs the collective rendezvous cost.
"""

import os
import sys
import numpy as np

import concourse.bass as bass
import concourse.mybir as mybir
import concourse.tile as tile
from concourse import bacc
from concourse import bass_utils

AF = mybir.ActivationFunctionType
ALU = mybir.AluOpType
F16 = mybir.dt.float16
F32 = mybir.dt.float32

EPS = 1e-07
N_CORES = 8
B = 32          # batch
J = 2048        # input capsules (total)
I = 64          # input capsule dim
K = 32          # output capsules
D = 32          # output capsule dim
JL = J // N_CORES          # 256 local input capsules
NPAIR = JL // 2            # 128 station pairs
DK = D * K                 # 1024
NC = DK // 128             # 8 dk-chunks of 128
EXP_SHIFT = -5.0           # softmax logit shift so exp() fits fp16 range


def build_program():
    """Build the SPMD bass program (same program on all 8 cores)."""
    nc = bacc.Bacc("TRN2", target_bir_lowering=False, debug=False,
                   enable_asserts=False, num_devices=N_CORES)

    xs = nc.dram_tensor("xs", [128, NPAIR, I], F16, kind="ExternalInput").ap()
    wt = nc.dram_tensor("wt", [NPAIR, 128, DK], F16, kind="ExternalInput").ap()
    mask = nc.dram_tensor("mask", [128, K], F16, kind="ExternalInput").ap()
    zsta = nc.dram_tensor("zsta", [128, 128], F16, kind="ExternalInput").ap()
    rep32 = nc.dram_tensor("rep32", [K, 128], F32, kind="ExternalInput").ap()
    crep = nc.dram_tensor("crep", [128, 4, 128], F16,
                          kind="ExternalInput").ap()
    out_d = nc.dram_tensor("out", [128, NC * B], F32,
                           kind="ExternalOutput").ap()

    with tile.TileContext(nc) as tc:
        _emit(tc, xs, wt, mask, zsta, rep32, crep, out_d)
    nc.compile()
    return nc


def _emit(tc, xs_d, wt_d, mask_d, zsta_d, rep32_d, crep_d, out_d):
    nc = tc.nc
    with (
        tc.tile_pool(name="hat", bufs=1) as hat_pool,
        tc.tile_pool(name="cst", bufs=1) as const_pool,
        tc.tile_pool(name="wld", bufs=3) as w_pool,
        tc.tile_pool(name="xld", bufs=2) as x_pool,
        tc.tile_pool(name="chp", bufs=2) as ch_pool,
        tc.tile_pool(name="chb", bufs=2) as chb_pool,
        tc.tile_pool(name="ep", bufs=2) as e_pool,
        tc.tile_pool(name="rzp", bufs=1) as rz_pool,
        tc.tile_pool(name="ckp", bufs=2) as ck_pool,
        tc.tile_pool(name="sm", bufs=1) as small_pool,
        tc.tile_pool(name="s16p", bufs=1) as s16_pool,
        tc.tile_pool(name="psA", bufs=2, space="PSUM") as psA_pool,
        tc.tile_pool(name="psR", bufs=6, space="PSUM") as psR_pool,
        tc.tile_pool(name="dram", bufs=8, space="DRAM") as dram_pool,
    ):
        # ---- constants ----
        mask_sb = const_pool.tile([128, K], F16, tag="mask")
        nc.sync.dma_start(mask_sb[:], mask_d)
        zsta_sb = const_pool.tile([128, 128], F16, tag="zsta")
        nc.sync.dma_start(zsta_sb[:], zsta_d)
        rep32_sb = const_pool.tile([K, 128], F32, tag="rep32")
        nc.sync.dma_start(rep32_sb[:], rep32_d)
        crep_sb = const_pool.tile([128, 4, 128], F16, tag="crep")
        nc.sync.dma_start(crep_sb[:], crep_d)
        ebias = const_pool.tile([128, 1], F32, tag="ebias")
        nc.vector.memset(ebias[:], EXP_SHIFT)
        mask32 = const_pool.tile([128, K], F32, tag="mask32")
        nc.vector.tensor_copy(mask32[:], mask_sb[:])

        # warm up the collective path early (absorbs CC rendezvous latency)
        warm_in = dram_pool.tile([128, K], F16, name="warm_in")
        warm_out = dram_pool.tile([128, K], F16, name="warm_out")
        nc.sync.dma_start(warm_in[:], mask_d)
        nc.gpsimd.collective_compute(
            "AllReduce", ALU.add,
            replica_groups=[list(range(N_CORES))],
            ins=[warm_in.opt()],
            outs=[warm_out.opt()],
        )

        # persistent tensors
        # hat3[p=(dm4,k), c, b, j] = hat[b, j, dk=c*128+p]  (d = 4c + dm4)
        hat3 = hat_pool.tile([128, NC, B, JL], F16, tag="hat")
        osta = const_pool.tile([128, NC, B, K], F16, tag="osta")
        o_acc = const_pool.tile([128, NC, B], F32, tag="oacc")
        o_acc16 = const_pool.tile([128, NC, B], F16, tag="oacc16")
        s0h = const_pool.tile([128, NC, B, 2], F32, tag="s0h")

        # ---- Pass A: hat = x @ W (W stationary), s0 = sum_j hat ----
        # Act does all psum->sbuf copies (GPSIMD can't read PSUM);
        # DVE does the s0 reduces under the DMA shadow.
        for pr in range(NPAIR):
            if pr % 8 == 0:
                xs_t = x_pool.tile([128, 8, I], F16, tag="xs",
                                   name=f"xs_{pr}")
                nc.sync.dma_start(xs_t[:], xs_d[:, pr:pr + 8, :])
            if pr % 2 == 0:
                wq2 = w_pool.tile([128, 2, DK], F16, tag="w", name=f"w_{pr}")
                nc.sync.dma_start(
                    wq2[:], wt_d[pr:pr + 2].rearrange("q p f -> p q f"))
            ps = psA_pool.tile([128, NC, 2 * B], F32, tag="psA",
                               name=f"psA_{pr}")
            for c in range(NC):
                nc.tensor.matmul(
                    ps[:, c, :],
                    lhsT=wq2[:, pr % 2, c * 128:(c + 1) * 128],
                    rhs=xs_t[:, pr % 8, :],
                    start=True, stop=True,
                )
            src = ps.rearrange("p c (b q) -> p c b q", q=2)
            dst = hat3[:, :, :, 2 * pr:2 * pr + 2]
            nc.scalar.copy(dst, src)
            # s0 partial reduces once each j-half is complete
            if pr == NPAIR // 2 - 1 or pr == NPAIR - 1:
                h = 0 if pr < NPAIR // 2 else 1
                for bb in range(16):
                    bs = 2 * bb
                    nc.vector.reduce_sum(
                        s0h[:, :, bs:bs + 2, h:h + 1],
                        hat3[:, :, bs:bs + 2, 128 * h:128 * (h + 1)],
                        axis=mybir.AxisListType.X,
                    )

        s16 = s16_pool.tile([128, NC * B], F16, tag="s16", name="s16_0")
        nc.vector.tensor_tensor(
            s16.rearrange("p (c b) -> p c b", c=NC),
            s0h[:, :, :, 0], s0h[:, :, :, 1], ALU.add,
        )

        # ---- routing iterations ----
        for r in range(3):
            # AllReduce the s partial sums (fp16, 64KB)
            ar_in = dram_pool.tile([128, NC * B], F16, name=f"ar_in{r}")
            ar_out = dram_pool.tile([128, NC * B], F16, name=f"ar_out{r}")
            nc.sync.dma_start(ar_in[:], s16[:])
            nc.gpsimd.collective_compute(
                "AllReduce", ALU.add,
                replica_groups=[list(range(N_CORES))],
                ins=[ar_in.opt()],
                outs=[ar_out.opt()],
            )
            s16b = s16_pool.tile([128, NC * B], F16, tag="s16b",
                                 name=f"s16b{r}")
            nc.sync.dma_start(s16b[:], ar_out[:])
            s_full = small_pool.tile([128, NC, B], F32, tag="sfull",
                                     name=f"sfull{r}")
            sfv = s_full.rearrange("p c b -> p (c b)")
            if r == 0:
                nc.vector.tensor_scalar_mul(sfv, s16b[:], 1.0 / K)
            else:
                nc.vector.tensor_copy(sfv, s16b[:])

            # squash scale: s2/(1+s2)/sqrt(s2+eps) per (k, b)
            sq = small_pool.tile([128, NC, B], F32, tag="sq", name=f"sq{r}")
            nc.scalar.square(sq[:], s_full[:])
            t4 = small_pool.tile([128, 4, B], F32, tag="t4", name=f"t4_{r}")
            nc.vector.tensor_tensor(t4[:], sq[:, 0:4, :], sq[:, 4:8, :],
                                    ALU.add)
            t2 = small_pool.tile([128, 2, B], F32, tag="t2", name=f"t2_{r}")
            nc.vector.tensor_tensor(t2[:], t4[:, 0:2, :], t4[:, 2:4, :],
                                    ALU.add)
            t1 = small_pool.tile([128, B], F32, tag="t1", name=f"t1_{r}")
            nc.vector.tensor_tensor(t1[:], t2[:, 0, :], t2[:, 1, :], ALU.add)
            # fold over the 4 dm4 partition groups via a delta-station matmul
            sqz = psR_pool.tile([128, JL], F32, tag="psr", name=f"sqz{r}")
            nc.tensor.matmul(sqz[0:K, 0:B], lhsT=mask32[:], rhs=t1[:],
                             start=True, stop=True)
            s2e = small_pool.tile([K, B], F32, tag="s2e", name=f"s2e{r}")
            nc.vector.tensor_scalar_add(s2e[:], sqz[0:K, 0:B], EPS)
            rt = small_pool.tile([K, B], F32, tag="rt", name=f"rt{r}")
            nc.scalar.sqrt(rt[:], s2e[:])
            den = small_pool.tile([K, B], F32, tag="den", name=f"den{r}")
            nc.vector.scalar_tensor_tensor(den[:], sqz[0:K, 0:B], 1.0, rt[:],
                                           ALU.add, ALU.mult)
            rden = small_pool.tile([K, B], F32, tag="rden", name=f"rden{r}")
            nc.vector.reciprocal_approx_fast(rden[:], den[:])
            scl = small_pool.tile([K, B], F32, tag="scl", name=f"scl{r}")
            nc.vector.tensor_tensor(scl[:], sqz[0:K, 0:B], rden[:], ALU.mult)
            # replicate scl to all 128 partitions via delta station
            sclp = psR_pool.tile([128, JL], F32, tag="psr", name=f"sclp{r}")
            nc.tensor.matmul(sclp[:, 0:B], lhsT=rep32_sb[:], rhs=scl[:],
                             start=True, stop=True)
            scl128 = small_pool.tile([128, B], F32, tag="sc128",
                                     name=f"sc128_{r}")
            nc.scalar.copy(scl128[:], sclp[:, 0:B])

            o_r = small_pool.tile([128, NC, B], F32, tag="or", name=f"or{r}")
            nc.vector.tensor_tensor(
                o_r[:],
                s_full[:],
                scl128[:, None, :].to_broadcast([128, NC, B]),
                ALU.mult,
            )

            if r == 2:
                nc.sync.dma_start(out_d, o_r.rearrange("p c b -> p (c b)"))
                break

            if r == 0:
                nc.vector.tensor_copy(o_acc[:], o_r[:])
            else:
                nc.vector.tensor_add(o_acc[:], o_acc[:], o_r[:])
            nc.scalar.copy(o_acc16[:], o_acc[:])
            # delta-masked stations: osta[p,c,b,k'] = (p%32==k') * O_acc[p,c,b]
            nc.gpsimd.tensor_tensor(
                osta[:],
                mask_sb[:, None, None, :].to_broadcast([128, NC, B, K]),
                o_acc16[:, :, :, None].to_broadcast([128, NC, B, K]),
                ALU.mult,
            )

            s16 = s16_pool.tile([128, NC * B], F16, tag="s16",
                                name=f"s16_{r + 1}")
            s32 = small_pool.tile([128, NC, B], F32, tag="s32",
                                  name=f"s32_{r}")

            # routing pass over 4-b blocks
            chi = 0
            for bb4 in range(8):
                b0 = 4 * bb4
                u_ps = psR_pool.tile([128, JL], F32, tag="psr",
                                     name=f"ups{r}_{bb4}")
                for c in range(NC):
                    for g in range(4):
                        nc.tensor.matmul(
                            u_ps[32 * g:32 * (g + 1), :],
                            lhsT=osta[:, c, b0 + g, :],
                            rhs=hat3[:, c, b0 + g, :],
                            start=(c == 0), stop=(c == NC - 1),
                            tile_position=(0, 32 * g),
                            skip_group_check=True,
                        )
                e16 = e_pool.tile([128, JL], F16, tag="e16",
                                  name=f"e{r}_{bb4}")
                nc.scalar.activation(e16[:], u_ps[:], AF.Exp, bias=ebias[:])
                z_ps = psR_pool.tile([128, JL], F32, tag="psr",
                                     name=f"z{r}_{bb4}")
                nc.tensor.matmul(z_ps[:], lhsT=zsta_sb[:], rhs=e16[:],
                                 start=True, stop=True)
                rz32 = rz_pool.tile([128, JL], F32, tag="rz32",
                                    name=f"rz32_{r}_{bb4}")
                nc.vector.reciprocal_approx_fast(rz32[:], z_ps[:])
                rz16 = rz_pool.tile([128, JL], F16, tag="rz16",
                                    name=f"rz16_{r}_{bb4}")
                nc.scalar.copy(rz16[:], rz32[:])
                # softmax weights, k-in-partition, aligned per b-group
                c_k4 = ck_pool.tile([128, JL], F16, tag="ck",
                                    name=f"ck{r}_{bb4}")
                for g in range(4):
                    nc.vector.tensor_tensor(
                        c_k4[32 * g:32 * (g + 1), :],
                        e16[32 * g:32 * (g + 1), :],
                        rz16[32 * g:32 * (g + 1), :],
                        ALU.mult,
                    )
                # replicate each b's [k, j] block to all 128 partitions (PE)
                cek = ck_pool.tile([128, 4, JL], F16, tag="cexp",
                                   name=f"cek{r}_{bb4}")
                for g in range(4):
                    ce = psR_pool.tile([128, JL], F32, tag="psr",
                                       name=f"ce{r}_{bb4}_{g}")
                    nc.tensor.matmul(ce[:], lhsT=crep_sb[:, g, :],
                                     rhs=c_k4[:], start=True, stop=True)
                    nc.scalar.copy(cek[:, g, :], ce[:])
                # ch = c*hat, then j-reduction: ping-pong halving tree with
                # fresh 3-dim-AP outputs (2x DVE mode), reduce_sum tail.
                for sb in range(2):
                    bs = b0 + 2 * sb
                    on_pool = chi % 5 == 4   # ~3 of 16 blocks on Pool
                    eng = nc.gpsimd if on_pool else nc.vector
                    chi += 1
                    ch = ch_pool.tile([128, NC, 2, JL], F16, tag="ch",
                                      name=f"ch{r}_{bs}")
                    chv = ch.rearrange("p c s j -> p (c s) j")
                    chb = chb_pool.tile([128, NC * 2, JL // 2], F16,
                                        tag="chb", name=f"chb{r}_{bs}")
                    eng.tensor_tensor(
                        ch[:],
                        hat3[:, :, bs:bs + 2, :],
                        cek[:, None, 2 * sb:2 * sb + 2, :].to_broadcast(
                            [128, NC, 2, JL]),
                        ALU.mult,
                    )
                    # L1..L4: 256 -> 16, alternating chb/chv outputs
                    eng.tensor_tensor(chb[:, :, 0:128], chv[:, :, 0:128],
                                      chv[:, :, 128:256], ALU.add)
                    eng.tensor_tensor(chv[:, :, 0:64], chb[:, :, 0:64],
                                      chb[:, :, 64:128], ALU.add)
                    eng.tensor_tensor(chb[:, :, 0:32], chv[:, :, 0:32],
                                      chv[:, :, 32:64], ALU.add)
                    eng.tensor_tensor(chv[:, :, 0:16], chb[:, :, 0:16],
                                      chb[:, :, 16:32], ALU.add)
                    if on_pool:
                        # Pool has no X-reduce; finish the tree
                        eng.tensor_tensor(chb[:, :, 0:8], chv[:, :, 0:8],
                                          chv[:, :, 8:16], ALU.add)
                        eng.tensor_tensor(chv[:, :, 0:4], chb[:, :, 0:4],
                                          chb[:, :, 4:8], ALU.add)
                        eng.tensor_tensor(chb[:, :, 0:2], chv[:, :, 0:2],
                                          chv[:, :, 2:4], ALU.add)
                        eng.tensor_tensor(
                            s32[:, :, bs:bs + 2],
                            chb[:, :, 0].rearrange("p (c s) -> p c s", c=NC),
                            chb[:, :, 1].rearrange("p (c s) -> p c s", c=NC),
                            ALU.add)
                    else:
                        nc.vector.reduce_sum(
                            s32[:, :, bs:bs + 2],
                            ch[:, :, :, 0:16],
                            axis=mybir.AxisListType.X)
            nc.scalar.copy(s16.rearrange("p (c b) -> p c b", c=NC), s32[:])


def pack_inputs(inputs, W):
    """Host-side shard + layout pack. Returns in_maps (one dict per core)."""
    mask = np.zeros((128, K), np.float16)
    mask[np.arange(128), np.arange(128) % K] = 1.0
    zsta = np.kron(np.eye(4, dtype=np.float16),
                   np.ones((32, 32), np.float16))
    # rep32[k, m] = (m%32 == k): replicates a [32, .] tile to 128 partitions
    rep32 = np.zeros((K, 128), np.float32)
    rep32[np.arange(128) % K, np.arange(128)] = 1.0
    # crep[p', g, m] = (p'//32 == g) & (p'%32 == m%32): selects b-group g's
    # [k, j] block and replicates it across the 4 dm4 partition groups
    crep = np.zeros((128, 4, 128), np.float16)
    pp = np.arange(128)
    for g in range(4):
        sel = (pp // 32 == g)
        for m in range(128):
            crep[sel & (pp % 32 == m % 32), g, m] = 1.0

    in_maps = []
    for c in range(N_CORES):
        jsl = slice(c * JL, (c + 1) * JL)
        # W: [K, J, D, I] -> [JL, I, D, K] -> [pair, (jp,i), (d,k)] fp16
        wc = np.ascontiguousarray(
            W[:, jsl].transpose(1, 3, 2, 0), dtype=np.float16
        )  # [JL, I, D, K]
        wt = wc.reshape(NPAIR, 2 * I, DK)

        # x stations: xs[p=(jp,i), pair, col=2b+jp] block-diag, partition-major
        xc = inputs[:, jsl, :]  # [B, JL, I]
        xt = np.ascontiguousarray(xc.transpose(1, 2, 0)).astype(np.float16)
        xs = np.zeros((NPAIR, 128, I), np.float16)
        xs[:, 0:I, 0::2] = xt[0::2]      # jp=0 rows, even cols
        xs[:, I:128, 1::2] = xt[1::2]    # jp=1 rows, odd cols
        xs2 = np.ascontiguousarray(xs.transpose(1, 0, 2))  # [128, NPAIR, I]
        in_maps.append({"xs": xs2, "wt": wt, "mask": mask, "zsta": zsta,
                        "rep32": rep32, "crep": crep})
    return in_maps


_CACHED_NC = None


def _install_ntff_hook():
    """Provide antenv.axon_hooks.get_axon_ntff_profile_hook when the agent
    image lacks it, by driving the injected libaxon_pjrt.so directly
    (mirrors trn_agent_boot._ntff_profile_via_ctypes)."""
    import types
    import ctypes
    import contextlib
    try:
        from antenv.axon_hooks import get_axon_ntff_profile_hook  # noqa: F401
        return True
    except ImportError:
        pass
    so_path = "/opt/axon/libaxon_pjrt.so"
    if not os.path.exists(so_path):
        return False
    lib = ctypes.CDLL(so_path)
    if not hasattr(lib, "axon_start_nrt_profile"):
        return False
    lib.axon_start_nrt_profile.argtypes = [
        ctypes.POINTER(ctypes.c_int64), ctypes.c_size_t]
    lib.axon_start_nrt_profile.restype = ctypes.c_int64
    lib.axon_stop_nrt_profile.argtypes = [ctypes.c_char_p]
    lib.axon_stop_nrt_profile.restype = ctypes.c_int64

    @contextlib.contextmanager
    def _hook(output_dir, device_ids):
        import jax
        jax.devices()
        if device_ids:
            ids = (ctypes.c_int64 * len(device_ids))(*device_ids)
            rc = lib.axon_start_nrt_profile(ids, len(device_ids))
        else:
            rc = lib.axon_start_nrt_profile(None, 0)
        if rc != 0:
            raise RuntimeError(f"axon_start_nrt_profile rc={rc}")
        try:
            yield
        finally:
            n = lib.axon_stop_nrt_profile(str(output_dir).encode())
            if n < 0:
                raise RuntimeError(f"axon_stop_nrt_profile rc={n}")

    import antenv
    mod = types.ModuleType("antenv.axon_hooks")
    mod.get_axon_ntff_profile_hook = lambda: _hook
    mod.set_axon_ntff_profile_hook = lambda h: None
    sys.modules["antenv.axon_hooks"] = mod
    antenv.axon_hooks = mod
    return True


def kernel(inputs, W):
    global _CACHED_NC
    inputs = np.asarray(inputs)
    W = np.asarray(W)
    if _CACHED_NC is None:
        _CACHED_NC = build_program()
    nc = _CACHED_NC
    in_maps = pack_inputs(inputs, W)
    trace = bool(int(os.environ.get("CAPS_TRACE", "0")))
    if trace:
        trace = _install_ntff_hook()
    res = bass_utils.run_bass_kernel_spmd(
        nc, in_maps, core_ids=list(range(N_CORES)), trace=trace,
    )
    kernel.last_results = res
    if trace and res.exec_time_ns is not None:
        print(f"HW exec time: {res.exec_time_ns} ns", file=sys.stderr)
        kernel.last_exec_time_ns = res.exec_time_ns
    out = res.results[0]["out"]  # [128 p=(dm4,k), NC*B] fp32 device layout
    a = out.reshape(4, K, NC, B)         # [dm4, k, c, b]; d = 4c + dm4
    return np.ascontiguousarray(
        a.transpose(3, 1, 2, 0).reshape(B, K, D)
    ).astype(np.float32)


kernel.last_exec_time_ns = None
kernel.last_results = None
